# revision 1
# baseline (speedup 1.0000x reference)
"""TRN2 Bass kernel for nn_BeyazKusAIAttention_36515811951168.

Key reduction: the reference applies softmax over a size-1 axis, which is
identically 1.0, so attention weights are exactly 1 and the module collapses
to
    y = (x @ Wv^T) @ Wfold^T,  with  Wfold = Wo.reshape(4096,4,1024).sum(1)
(q/rope/scores/mask are dead code; `out` is v tiled over the 4 heads, and the
o-projection of the tiled v folds head-wise into Wfold).  This is a 5x FLOP
reduction vs the reference graph.

Execution: data-parallel over the 16384 = batch*seq rows across 8 NeuronCores
(no collectives).  All matmul operands fp16 (fp32 PSUM accumulation); y is
written to DRAM as fp16 and upcast on host.  Measured end-to-end relative
error vs the fp32 reference is ~4.6e-4.

Per-core program (R = 2048 rows, 4 chunks of 512), tensor-engine-bound at
~218 ns per 512-wide matmul (2048 matmuls/core):
  MM1: v^T = Wv @ x^T  - Wv^T resident in SBUF (32 [128,1024] f16 tiles),
       x^T chunk tiles streamed, K=4096 accumulated; the 8 v^T output tiles
       are computed in two 4-bank halves so the PSUM->SBUF evictions (DVE)
       hide under the other half's streaming.
  MM2: y = v @ Wfold^T - Wfold^T resident in SBUF (8 MB f16), v^T tiles
       stationary from SBUF, groups tag-rotate the 8 PSUM banks with the
       contraction (k2) loop ascending so MM2 starts before all v tiles
       are evicted.
  DMA emission order matters (single FIFO queue): chunk-0 Wv+x tiles first
  (matmuls start at ~2 us instead of waiting 50 us behind the Wfold load),
  Wfold interleaved with chunk-1 x after chunk-0's MM1.

Host-side layouts (partition dim = contraction dim for both matmuls):
  xt [32,128,R]: xt[k,p,r] = x[row r, dim 128k+p]     (transposed shard)
  wvt[32,128,1024]: wvt[k,p,m] = Wv[m, 128k+p]
  wft[8,128,4096]:  wft[k,p,n] = Wfold[n, 128k+p]
  y  [R/128,128,4096] (f16): y[t,p,n] = out[row 128t+p, n]
"""
import numpy as np
import concourse.bass as bass
from concourse import bacc
import concourse.mybir as mybir
from concourse.tile import TileContext
from concourse.bass_utils import run_bass_kernel_spmd

DIM = 4096
KV = 1024
N_CORES = 8
ROWS_TOTAL = 4 * 4096
ROWS = ROWS_TOTAL // N_CORES   # 2048
KT1 = DIM // 128               # 32 k-tiles, phase 1
MT1 = KV // 128                # 8 vcol tiles
KT2 = KV // 128                # 8 k-tiles, phase 2
NC2 = DIM // 512               # 8 ycol chunks
CH = 512                       # row-chunk width

_nc_cache = {}


def _build(rows=ROWS):
    nch = rows // CH
    f32 = mybir.dt.float32
    f16 = mybir.dt.float16

    nc = bacc.Bacc(None, target_bir_lowering=False)
    XT = nc.dram_tensor("xt", [KT1, 128, rows], f16, kind="ExternalInput")
    WVT = nc.dram_tensor("wvt", [KT1, 128, KV], f16, kind="ExternalInput")
    WFT = nc.dram_tensor("wft", [KT2, 128, DIM], f16, kind="ExternalInput")
    Y = nc.dram_tensor("y", [rows // 128, 128, DIM], f16,
                       kind="ExternalOutput")

    with TileContext(nc) as tc:
        with (
            tc.tile_pool(name="wf", bufs=1) as wfpool,
            tc.tile_pool(name="wv", bufs=1) as wvpool,
            tc.tile_pool(name="xts", bufs=32) as xtpool,
            tc.tile_pool(name="vss", bufs=1) as vspool,
            tc.tile_pool(name="yst", bufs=16) as ypool,
            tc.tile_pool(name="ps", bufs=1, space="PSUM") as pspool,
        ):
            def load_xt(rc, k):
                xtk = xtpool.tile([128, CH], f16, tag="xt",
                                  name=f"xt_{rc}_{k}")
                nc.sync.dma_start(xtk[:], XT[k, :, rc * CH:(rc + 1) * CH])
                return xtk

            # chunk-0 weights+x first so MM1 starts immediately;
            # Wv on the ACT HWDGE ring so it streams parallel to x (SP ring)
            wv = []
            xts = [load_xt(0, 0)]
            for k in range(KT1):
                wvk = wvpool.tile([128, KV], f16, tag=f"wv{k}",
                                  name=f"wv{k}")
                nc.scalar.dma_start(wvk[:], WVT[k])
                wv.append(wvk)
                if k < KT1 - 1:
                    xts.append(load_xt(0, k + 1))
            wf = []

            def emit_wf_and_xt1():
                xt1 = []
                for n in range(NC2):
                    wfn = wfpool.tile([128, KT2, 512], f16,
                                      tag=f"wf{n}", name=f"wf{n}")
                    for k in range(KT2):
                        nc.sync.dma_start(
                            wfn[:, k, :], WFT[k, :, n * 512:(n + 1) * 512])
                    wf.append(wfn)
                    if nch > 1:
                        for k in range(n * 4, n * 4 + 4):
                            xt1.append(load_xt(1, k))
                return xt1

            xt_next = None
            for rc in range(nch):
                xtc = xts if rc == 0 else xt_next
                vs = [None] * MT1
                # MM1 in two 4-bank halves; evictions hide under streaming
                for mh in range(2):
                    ps1 = [pspool.tile([128, CH], f32,
                                       tag=f"P{mh * 4 + j}",
                                       name=f"ps1_{rc}_{mh}_{j}")
                           for j in range(4)]
                    for k in range(KT1):
                        for j in range(4):
                            m = mh * 4 + j
                            nc.tensor.matmul(
                                ps1[j][:], wv[k][:, m * 128:(m + 1) * 128],
                                xtc[k][:], start=(k == 0),
                                stop=(k == KT1 - 1))
                    for j in range(4):
                        m = mh * 4 + j
                        v = vspool.tile([128, CH], f16, tag=f"vs{m}",
                                        name=f"vs_{rc}_{m}")
                        nc.vector.tensor_copy(v[:], ps1[j][:])
                        vs[m] = v
                if rc == 0:
                    xt_next = emit_wf_and_xt1()
                elif rc < nch - 1:
                    xt_next = [load_xt(rc + 1, k) for k in range(KT1)]
                for g, (n, sub) in enumerate(
                        (n, s) for n in range(NC2)
                        for s in range(CH // 128)):
                    ps2 = pspool.tile([128, 512], f32, tag=f"P{g % 8}",
                                      name=f"ps2_{rc}_{g}")
                    for k2 in range(KT2):
                        nc.tensor.matmul(
                            ps2[:], vs[k2][:, sub * 128:(sub + 1) * 128],
                            wf[n][:, k2, :],
                            start=(k2 == 0), stop=(k2 == KT2 - 1))
                    ys = ypool.tile([128, 512], f16, tag="ys",
                                    name=f"ys_{rc}_{g}")
                    nc.vector.tensor_copy(ys[:], ps2[:])
                    # y store on the ACT HWDGE ring so output DMAs aren't
                    # queued behind input loads (SP ring is FIFO)
                    nc.scalar.dma_start(
                        Y[rc * (CH // 128) + sub, :,
                          n * 512:(n + 1) * 512], ys[:])
    nc.compile()
    return nc


def kernel(x, Wq, Wk, Wv, Wo, mask):
    x = np.asarray(x)
    Wv = np.asarray(Wv, dtype=np.float32)
    Wo = np.asarray(Wo, dtype=np.float32)
    B, S, D = x.shape
    assert D == DIM and B * S == ROWS_TOTAL

    # host-side relayout: transpose x once, fold Wo over heads
    x2 = np.ascontiguousarray(
        x.reshape(ROWS_TOTAL, DIM).T).astype(np.float16)
    xt_all = x2.reshape(KT1, 128, ROWS_TOTAL)
    wvt = np.ascontiguousarray(Wv.T).astype(np.float16).reshape(KT1, 128, KV)
    wfold = Wo.reshape(DIM, 4, KV).sum(axis=1)
    wft = np.ascontiguousarray(
        wfold.T).astype(np.float16).reshape(KT2, 128, DIM)

    in_maps = []
    for c in range(N_CORES):
        in_maps.append({
            "xt": np.ascontiguousarray(
                xt_all[:, :, c * ROWS:(c + 1) * ROWS]),
            "wvt": wvt,
            "wft": wft,
        })

    if "nc" not in _nc_cache:
        _nc_cache["nc"] = _build()
    nc = _nc_cache["nc"]

    # transient NRT device errors (e.g. NRT_EXEC_UNIT_UNRECOVERABLE right
    # after another process released the cores) succeed on retry
    last_err = None
    for _attempt in range(3):
        try:
            results = run_bass_kernel_spmd(
                nc, in_maps, core_ids=list(range(N_CORES))).results
            break
        except Exception as e:  # noqa: BLE001
            last_err = e
    else:
        raise last_err
    shards = [r["y"].reshape(ROWS, DIM) for r in results]
    out = np.concatenate(shards, axis=0).reshape(B, S, DIM)
    return out.astype(np.float32)



# revision 2
# speedup vs baseline: 1.0222x; 1.0222x over previous
"""TRN2 Bass kernel for nn_BeyazKusAIAttention_36515811951168.

Key reduction: the reference applies softmax over a size-1 axis, which is
identically 1.0, so attention weights are exactly 1 and the module collapses
to
    y = (x @ Wv^T) @ Wfold^T,  with  Wfold = Wo.reshape(4096,4,1024).sum(1)
(q/rope/scores/mask are dead code; `out` is v tiled over the 4 heads, and the
o-projection of the tiled v folds head-wise into Wfold).  This is a 5x FLOP
reduction vs the reference graph.

Execution: data-parallel over the 16384 = batch*seq rows across 8 NeuronCores
(no collectives).  All matmul operands fp16 (fp32 PSUM accumulation); y is
written to DRAM as fp16 and upcast on host.  Measured end-to-end relative
error vs the fp32 reference is ~4.6e-4.

Per-core program (R = 2048 rows, 4 chunks of 512). The PE column-stream is
the roofline (~1.22 cycles/column measured; 2048 matmuls x 512 cols); the
schedule keeps PE busy at ~98%:
  MM1: v^T = Wv @ x^T  - Wv^T resident in SBUF, x^T chunk tiles streamed,
       K=4096 accumulated.  Chunk 0 runs a single 8-PSUM-bank pass over k
       (halving the burst DMA demand while wv+xt stream in); later chunks
       use two 4-bank halves so PSUM->SBUF evictions hide under compute.
  MM2: y = v @ Wfold^T - Wfold^T resident in SBUF, v^T tiles stationary,
       groups tag-rotate the 8 PSUM banks, k2 ascending so MM2 starts
       before all v evictions land.
  Evictions alternate DVE / Activation engines so neither serializes the
  MM1->MM2 boundary. The ACT Copy table is pre-warmed off the critical path.
  DMA rings: wv + ys on the ACT HWDGE ring, xt + wf on the SP ring (chunk-0
  starvation is ring-balance-sensitive).  wv[0] is split in halves so the
  first matmul waits on 128KB, the last y store is split across both rings
  to shorten the tail drain.

Host-side layouts (partition dim = contraction dim for both matmuls):
  xt [32,128,R]: xt[k,p,r] = x[row r, dim 128k+p]     (transposed shard)
  wvt[32,128,1024]: wvt[k,p,m] = Wv[m, 128k+p]
  wft[8,128,4096]:  wft[k,p,n] = Wfold[n, 128k+p]
  y  [R/128,128,4096] (f16): y[t,p,n] = out[row 128t+p, n]
"""
import numpy as np
import concourse.bass as bass
from concourse import bacc
import concourse.mybir as mybir
from concourse.tile import TileContext
from concourse.bass_utils import run_bass_kernel_spmd

DIM = 4096
KV = 1024
N_CORES = 8
ROWS_TOTAL = 4 * 4096
ROWS = ROWS_TOTAL // N_CORES   # 2048
KT1 = DIM // 128               # 32 k-tiles, phase 1
MT1 = KV // 128                # 8 vcol tiles
KT2 = KV // 128                # 8 k-tiles, phase 2
NC2 = DIM // 512               # 8 ycol chunks
CH = 512                       # row-chunk width

_nc_cache = {}


def _build(rows=ROWS):
    nch = rows // CH
    f32 = mybir.dt.float32
    f16 = mybir.dt.float16

    nc = bacc.Bacc(None, target_bir_lowering=False)
    XT = nc.dram_tensor("xt", [KT1, 128, rows], f16, kind="ExternalInput")
    WVT = nc.dram_tensor("wvt", [KT1, 128, KV], f16, kind="ExternalInput")
    WFT = nc.dram_tensor("wft", [KT2, 128, DIM], f16, kind="ExternalInput")
    Y = nc.dram_tensor("y", [rows // 128, 128, DIM], f16,
                       kind="ExternalOutput")

    with TileContext(nc) as tc:
        with (
            tc.tile_pool(name="wf", bufs=1) as wfpool,
            tc.tile_pool(name="wv", bufs=1) as wvpool,
            tc.tile_pool(name="xts", bufs=32) as xtpool,
            tc.tile_pool(name="vss", bufs=1) as vspool,
            tc.tile_pool(name="yst", bufs=16) as ypool,
            tc.tile_pool(name="dumm", bufs=1) as dpool,
            tc.tile_pool(name="ps", bufs=1, space="PSUM") as pspool,
        ):
            def load_xt(rc, k):
                xtk = xtpool.tile([128, CH], f16, tag="xt",
                                  name=f"xt_{rc}_{k}")
                nc.sync.dma_start(xtk[:], XT[k, :, rc * CH:(rc + 1) * CH])
                return xtk

            # chunk-0 weights+x: wv on ACT ring (wv[0] split so the first
            # matmul waits on 128KB), xt streamed on the SP ring
            wv = []
            xts = [load_xt(0, 0)]
            for k in range(KT1):
                wvk = wvpool.tile([128, KV], f16, tag=f"wv{k}",
                                  name=f"wv{k}")
                if k == 0:
                    nc.scalar.dma_start(wvk[:, :512], WVT[0, :, :512])
                    nc.scalar.dma_start(wvk[:, 512:], WVT[0, :, 512:])
                else:
                    nc.scalar.dma_start(wvk[:], WVT[k])
                wv.append(wvk)
                if k < KT1 - 1:
                    xts.append(load_xt(0, k + 1))
            # pre-warm the ACT Copy table behind the wv DMAs on ACT SEQ,
            # long before the first ACT eviction
            dm = dpool.tile([128, 1], f16, tag="dm", name="dm")
            nc.vector.memset(dm[:], 0)
            dm2 = dpool.tile([128, 1], f16, tag="dm2", name="dm2")
            nc.scalar.copy(dm2[:], dm[:])
            wf = []

            def emit_wf_and_xt1():
                xt1 = []
                for n in range(NC2):
                    wfn = wfpool.tile([128, KT2, 512], f16,
                                      tag=f"wf{n}", name=f"wf{n}")
                    for k in range(KT2):
                        nc.sync.dma_start(
                            wfn[:, k, :], WFT[k, :, n * 512:(n + 1) * 512])
                    wf.append(wfn)
                    if nch > 1:
                        for k in range(n * 4, n * 4 + 4):
                            xt1.append(load_xt(1, k))
                return xt1

            def evict(dst, src, alt):
                if alt:
                    nc.scalar.copy(dst, src)
                else:
                    nc.vector.tensor_copy(dst, src)

            xt_next = None
            for rc in range(nch):
                xtc = xts if rc == 0 else xt_next
                vs = [None] * MT1
                if rc == 0:
                    # single 8-bank pass: halves the per-k-tile DMA demand
                    # while wv/xt stream in
                    ps1 = [pspool.tile([128, CH], f32, tag=f"P{m}",
                                       name=f"ps1_{rc}_{m}")
                           for m in range(MT1)]
                    for k in range(KT1):
                        for m in range(MT1):
                            nc.tensor.matmul(
                                ps1[m][:], wv[k][:, m * 128:(m + 1) * 128],
                                xtc[k][:], start=(k == 0),
                                stop=(k == KT1 - 1))
                    for m in range(MT1):
                        v = vspool.tile([128, CH], f16, tag=f"vs{m}",
                                        name=f"vs_{rc}_{m}")
                        evict(v[:], ps1[m][:], m % 2 == 1)
                        vs[m] = v
                else:
                    # two 4-bank halves; evictions hide under streaming
                    for mh in range(2):
                        ps1 = [pspool.tile([128, CH], f32,
                                           tag=f"P{mh * 4 + j}",
                                           name=f"ps1_{rc}_{mh}_{j}")
                               for j in range(4)]
                        for k in range(KT1):
                            for j in range(4):
                                m = mh * 4 + j
                                nc.tensor.matmul(
                                    ps1[j][:],
                                    wv[k][:, m * 128:(m + 1) * 128],
                                    xtc[k][:], start=(k == 0),
                                    stop=(k == KT1 - 1))
                        for j in range(4):
                            m = mh * 4 + j
                            v = vspool.tile([128, CH], f16, tag=f"vs{m}",
                                            name=f"vs_{rc}_{m}")
                            evict(v[:], ps1[j][:], j % 2 == 1)
                            vs[m] = v
                if rc == 0:
                    xt_next = emit_wf_and_xt1()
                elif rc < nch - 1:
                    xt_next = [load_xt(rc + 1, k) for k in range(KT1)]
                lastc = rc == nch - 1
                for g, (n, sub) in enumerate(
                        (n, s) for n in range(NC2)
                        for s in range(CH // 128)):
                    ps2 = pspool.tile([128, 512], f32, tag=f"P{g % 8}",
                                      name=f"ps2_{rc}_{g}")
                    for k2 in range(KT2):
                        nc.tensor.matmul(
                            ps2[:], vs[k2][:, sub * 128:(sub + 1) * 128],
                            wf[n][:, k2, :],
                            start=(k2 == 0), stop=(k2 == KT2 - 1))
                    ys = ypool.tile([128, 512], f16, tag="ys",
                                    name=f"ys_{rc}_{g}")
                    yrow = rc * (CH // 128) + sub
                    if lastc and g == NC2 * (CH // 128) - 1:
                        # split the final store across engines+rings to
                        # shorten the tail drain
                        nc.vector.tensor_copy(ys[:, :256], ps2[:, :256])
                        nc.scalar.copy(ys[:, 256:], ps2[:, 256:])
                        nc.scalar.dma_start(
                            Y[yrow, :, n * 512:n * 512 + 256], ys[:, :256])
                        nc.sync.dma_start(
                            Y[yrow, :, n * 512 + 256:(n + 1) * 512],
                            ys[:, 256:])
                    else:
                        nc.vector.tensor_copy(ys[:], ps2[:])
                        # y stores on ACT (chunks 0-2; SP is prefetching xt)
                        # and alternate ACT/SP in the last chunk (SP idle)
                        dma = (nc.sync.dma_start if lastc and g % 2 == 1
                               else nc.scalar.dma_start)
                        dma(Y[yrow, :, n * 512:(n + 1) * 512], ys[:])
    nc.compile()
    return nc


def kernel(x, Wq, Wk, Wv, Wo, mask):
    x = np.asarray(x)
    Wv = np.asarray(Wv, dtype=np.float32)
    Wo = np.asarray(Wo, dtype=np.float32)
    B, S, D = x.shape
    assert D == DIM and B * S == ROWS_TOTAL

    # host-side relayout: transpose x once, fold Wo over heads
    x2 = np.ascontiguousarray(
        x.reshape(ROWS_TOTAL, DIM).T).astype(np.float16)
    xt_all = x2.reshape(KT1, 128, ROWS_TOTAL)
    wvt = np.ascontiguousarray(Wv.T).astype(np.float16).reshape(KT1, 128, KV)
    wfold = Wo.reshape(DIM, 4, KV).sum(axis=1)
    wft = np.ascontiguousarray(
        wfold.T).astype(np.float16).reshape(KT2, 128, DIM)

    in_maps = []
    for c in range(N_CORES):
        in_maps.append({
            "xt": np.ascontiguousarray(
                xt_all[:, :, c * ROWS:(c + 1) * ROWS]),
            "wvt": wvt,
            "wft": wft,
        })

    if "nc" not in _nc_cache:
        _nc_cache["nc"] = _build()
    nc = _nc_cache["nc"]

    # transient NRT device errors (e.g. NRT_EXEC_UNIT_UNRECOVERABLE right
    # after another process released the cores) succeed on retry
    last_err = None
    for _attempt in range(3):
        try:
            results = run_bass_kernel_spmd(
                nc, in_maps, core_ids=list(range(N_CORES))).results
            break
        except Exception as e:  # noqa: BLE001
            last_err = e
    else:
        raise last_err
    shards = [r["y"].reshape(ROWS, DIM) for r in results]
    out = np.concatenate(shards, axis=0).reshape(B, S, DIM)
    return out.astype(np.float32)
